# revision 19
# baseline (speedup 1.0000x reference)
"""Trainium2 Bass kernel for nn_BasicBlockBi (TBN basic block, 2x ternary-binary conv).

Strategy: data-parallel over batch (4 images per core on 8 cores).
  - BN + ternarize thresholds for block 1 are folded on host into per-channel
    compare thresholds (delta1 computed on host from the full input).
  - Ternary activations {-1,0,+1} and binary sign weights are exact in bf16,
    so convs run as 3x3-tap shifted matmuls accumulating exact integers in PSUM;
    the per-output-channel alpha scale + shortcut add are fused in one DVE op.
  - delta2 = 0.7*mean|bn2(h)| needs a global mean over the full batch: per-core
    partial sums are AllReduced across the 8 cores in-kernel.
"""

import os
import sys

for _p in ("/opt/trn_rl_repo", "/root/.axon_site/_ro/trn_rl_repo"):
    if os.path.isdir(_p) and _p not in sys.path:
        sys.path.append(_p)

import numpy as np

import concourse.bacc as bacc
import concourse.bass as bass
import concourse.tile as tile
from concourse import bass_isa, mybir
from concourse.bass_utils import run_bass_kernel_spmd

B, C, H, W = 32, 256, 32, 32
HW = H * W
NCORES = 8
BL = B // NCORES          # images per core
CCH = C // 128            # channel chunks of 128
PW = W + 2                # padded row width
PBUF = 1 + PW * PW        # lead zero + 34x34 padded plane
PBUF_AL = PBUF + 3        # tail pad so over-slices stay in bounds
EPS = 1e-5
FRAC = 0.7

QMODE = "fp8"             # "fp8" (DoubleRow, 2x PE throughput) or "bf16"
QDT = mybir.dt.float8e4 if QMODE == "fp8" else mybir.dt.bfloat16  # exact for {-2..2}
SDT = mybir.dt.bfloat16   # Sign() activation output dtype

AOP = mybir.AluOpType
AFT = mybir.ActivationFunctionType
F32 = mybir.dt.float32

# vecs rows
V_NT1HI, V_NT1LO, V_A1, V_A2, V_S2, V_B2, V_BRS2, V_KRS2, V_NKRS2 = range(9)
NVEC = 9

TRACE = False
LAST_RESULT = None

_cache: dict = {}


def _ternarize(nc, neg_hi, neg_lo, src, qtile, tmp_pool):
    """qtile = sign(src - hi) + sign(src - lo) in {-2..2}; /2 is folded into alpha.

    Two Scalar-engine Sign activations (per-partition bias APs) + one contiguous
    DVE add. Exact in bf16.
    """
    a = tmp_pool.tile([128, HW], SDT, tag="t1")
    b = tmp_pool.tile([128, HW], SDT, tag="t2")
    nc.scalar.activation(a, src, AFT.Sign, bias=neg_hi, scale=1.0)
    nc.scalar.activation(b, src, AFT.Sign, bias=neg_lo, scale=1.0)
    nc.vector.tensor_tensor(qtile, a, b, AOP.add)


def _zero_all(nc, qtile):
    nc.vector.memset(qtile[:], 0.0)


def _pad_interior(qtile):
    return qtile[:, 1 + PW : 1 + PW + 32 * PW].rearrange(
        "p (r c) -> p r c", c=PW
    )[:, :, 0:32]


def _conv_matmuls(nc, psum_tile, wtiles, qp, co, half):
    """Accumulate the shifted-tap matmuls for one (co chunk, row half)."""
    idx = 0
    if QMODE == "fp8":
        for kh in range(3):
            for kw in range(3):
                off = PW * (16 * half + kh) + kw
                rhs = qp[:, :, off : off + PW * 16].rearrange(
                    "p t (r c) -> p t r c", c=PW
                )[:, :, :, 0:32]
                nc.tensor.matmul(
                    psum_tile,
                    lhsT=wtiles[kh * 3 + kw][:, :, co * 128 : (co + 1) * 128],
                    rhs=rhs,
                    start=(idx == 0),
                    stop=(idx == 8),
                    perf_mode=mybir.MatmulPerfMode.DoubleRow,
                )
                idx += 1
    else:
        for kh in range(3):
            for kw in range(3):
                off = PW * (16 * half + kh) + kw
                for ci in range(CCH):
                    rhs = qp[ci][:, off : off + PW * 16].rearrange(
                        "p (r c) -> p r c", c=PW
                    )[:, :, 0:32]
                    nc.tensor.matmul(
                        psum_tile,
                        lhsT=wtiles[kh * 3 + kw, ci][:, co * 128 : (co + 1) * 128],
                        rhs=rhs,
                        start=(idx == 0),
                        stop=(idx == 17),
                    )
                    idx += 1


def _build():
    if "nc" in _cache:
        return _cache["nc"]

    nc = bacc.Bacc("TRN2", num_devices=NCORES)

    x_in = nc.dram_tensor("x", (BL, CCH, 128, HW), F32, kind="ExternalInput")
    # fp8: [tap, k, ci, co] (DoubleRow pairs ci along dim2); bf16: [tap, ci, k, co]
    wshape = (9, 128, CCH, C) if QMODE == "fp8" else (9, CCH, 128, C)
    w1t = nc.dram_tensor("w1t", wshape, QDT, kind="ExternalInput")
    w2t = nc.dram_tensor("w2t", wshape, QDT, kind="ExternalInput")
    vecs = nc.dram_tensor("vecs", (NVEC, CCH, 128, 1), F32, kind="ExternalInput")
    out_d = nc.dram_tensor("out", (BL, CCH, 128, HW), F32, kind="ExternalOutput")
    cc_in = nc.dram_tensor("cc_in", (128, 1), F32)
    cc_out = nc.dram_tensor("cc_out", (128, 1), F32, addr_space="Shared")
    dsc = nc.dram_tensor("dsc", (1, 1), F32)

    with tile.TileContext(nc) as tc:
        with (
            tc.tile_pool(name="consts", bufs=1) as consts,
            tc.tile_pool(name="persist", bufs=1) as persist,
            tc.tile_pool(name="tmp", bufs=3) as tmp,
            tc.tile_pool(name="epi", bufs=4) as epi,
            tc.tile_pool(name="psum", bufs=8, space="PSUM") as psum,
        ):
            # ---- constants (one batched DMA) ----
            vtile = consts.tile([128, NVEC, CCH], F32, tag="vecs")
            nc.sync.dma_start(out=vtile, in_=vecs[:].rearrange("v c p one -> p v (c one)"))
            vt = {}
            for i in range(NVEC):
                for ci in range(CCH):
                    vt[i, ci] = vtile[:, i, ci : ci + 1]
            ones128 = consts.tile([128, 128], F32, tag="ones128")
            nc.vector.memset(ones128[:], 1.0)

            xims = {}
            for n in range(BL):
                xim_t = persist.tile([128, CCH, HW], F32, tag=f"x{n}")
                xims[n] = xim_t
            nc.sync.dma_start(out=xims[0], in_=x_in[0].rearrange("c p f -> p c f"))

            w1s, w2s = {}, {}
            if QMODE == "fp8":
                wa = consts.tile([128, 9, CCH, C], QDT, tag="w1all")
                nc.sync.dma_start(out=wa, in_=w1t[:].rearrange("t k c f -> k t c f"))
                wb = consts.tile([128, 9, CCH, C], QDT, tag="w2all")
                nc.sync.dma_start(out=wb, in_=w2t[:].rearrange("t k c f -> k t c f"))
                for tap in range(9):
                    w1s[tap] = wa[:, tap]
                    w2s[tap] = wb[:, tap]
            else:
                for tap in range(9):
                    for ci in range(CCH):
                        a = consts.tile([128, C], QDT, tag=f"w1_{tap}_{ci}")
                        nc.sync.dma_start(out=a, in_=w1t[tap, ci])
                        w1s[tap, ci] = a
                        b = consts.tile([128, C], QDT, tag=f"w2_{tap}_{ci}")
                        nc.sync.dma_start(out=b, in_=w2t[tap, ci])
                        w2s[tap, ci] = b

            partials = consts.tile([128, BL * CCH * 2], F32, tag="partials")

            def make_qpads(prefix):
                pads = {}
                for par in range(2):
                    for ci_or_all in ([None] if QMODE == "fp8" else range(CCH)):
                        if QMODE == "fp8":
                            qq = consts.tile([128, CCH, PBUF_AL], QDT, tag=f"{prefix}{par}")
                        else:
                            qq = consts.tile([128, PBUF_AL], QDT, tag=f"{prefix}{par}_{ci_or_all}")
                        _zero_all(nc, qq)
                        pads[par, ci_or_all] = qq
                return pads

            qpadsA = make_qpads("qpA")
            qpadsB = make_qpads("qpB")

            xt, ht = {}, {}

            # ---------- phase A: block 1 + |bn2(h)| partial sums ----------
            pcol = 0
            for n in range(BL):
                xim = xims[n]
                if n > 0:
                    nc.sync.dma_start(out=xim, in_=x_in[n].rearrange("c p f -> p c f"))
                for ci in range(CCH):
                    xt[n, ci] = xim[:, ci, :]

                if QMODE == "fp8":
                    qp = qpadsA[n % 2, None]
                else:
                    qp = {ci: qpadsA[n % 2, ci] for ci in range(CCH)}
                qf = tmp.tile([128, CCH, HW], QDT, tag="qf")
                for ci in range(CCH):
                    _ternarize(nc, vt[V_NT1HI, ci], vt[V_NT1LO, ci], xt[n, ci], qf[:, ci, :], tmp)
                for ci in range(CCH):
                    dstp = _pad_interior(qp[:, ci] if QMODE == "fp8" else qp[ci])
                    nc.sync.dma_start(
                        out=dstp,
                        in_=qf[:, ci, :].rearrange("p (r c) -> p r c", c=32),
                    )

                for co in range(CCH):
                    htile = persist.tile([128, HW], F32, tag=f"h{n}_{co}")
                    ht[n, co] = htile
                    for half in range(2):
                        ps = psum.tile([128, 512], F32, tag="ps")
                        _conv_matmuls(nc, ps, w1s, qp, co, half)
                        sl = slice(half * 512, (half + 1) * 512)
                        # h = alpha1 * conv + x   (one DVE op)
                        nc.vector.scalar_tensor_tensor(
                            out=htile[:, sl],
                            in0=ps,
                            scalar=vt[V_A1, co],
                            in1=xt[n, co][:, sl],
                            op0=AOP.mult,
                            op1=AOP.add,
                        )
                        # |bn2(h)| with per-partition running sum for delta2
                        zabs = tmp.tile([128, 512], F32, tag="zabs")
                        nc.scalar.activation(
                            out=zabs,
                            in_=htile[:, sl],
                            func=AFT.Abs,
                            bias=vt[V_B2, co],
                            scale=vt[V_S2, co],
                            accum_out=partials[:, pcol : pcol + 1],
                        )
                        pcol += 1

            # ---------- delta2 via cross-core AllReduce ----------
            ptot = consts.tile([128, 1], F32, tag="ptot")
            nc.vector.tensor_reduce(ptot, partials, axis=mybir.AxisListType.X, op=AOP.add)
            nc.sync.dma_start(out=cc_in[:], in_=ptot)
            nc.gpsimd.collective_compute(
                "AllReduce",
                AOP.add,
                replica_groups=[list(range(NCORES))],
                ins=[cc_in[:]],
                outs=[cc_out[:]],
            )
            # broadcast-sum the 128 AllReduced per-partition values to every
            # partition with one ones-matmul (PE is idle here), then form the
            # negated sign-bias thresholds in one fused DVE op each:
            #   -t2hi = tot*(-k/s2) + b2/s2 ;  -t2lo = tot*(k/s2) + b2/s2
            red = consts.tile([128, 1], F32, tag="red")
            nc.sync.dma_start(out=red, in_=cc_out[:])
            d2tot_bank = psum.tile([128, 512], F32, tag="ps")
            d2tot = d2tot_bank[:, 0:1]
            nc.tensor.matmul(d2tot, lhsT=ones128, rhs=red, start=True, stop=True)
            nt2hi, nt2lo = {}, {}
            for ci in range(CCH):
                thi = consts.tile([128, 1], F32, tag=f"nt2hi{ci}")
                nc.vector.scalar_tensor_tensor(
                    out=thi, in0=d2tot, scalar=vt[V_NKRS2, ci], in1=vt[V_BRS2, ci],
                    op0=AOP.mult, op1=AOP.add)
                nt2hi[ci] = thi
                tlo = consts.tile([128, 1], F32, tag=f"nt2lo{ci}")
                nc.vector.scalar_tensor_tensor(
                    out=tlo, in0=d2tot, scalar=vt[V_KRS2, ci], in1=vt[V_BRS2, ci],
                    op0=AOP.mult, op1=AOP.add)
                nt2lo[ci] = tlo

            # ---------- phase B: block 2 ----------
            for n in range(BL):
                if QMODE == "fp8":
                    qp = qpadsB[n % 2, None]
                else:
                    qp = {ci: qpadsB[n % 2, ci] for ci in range(CCH)}
                qf = tmp.tile([128, CCH, HW], QDT, tag="qf")
                for ci in range(CCH):
                    _ternarize(nc, nt2hi[ci], nt2lo[ci], ht[n, ci], qf[:, ci, :], tmp)
                for ci in range(CCH):
                    dstp = _pad_interior(qp[:, ci] if QMODE == "fp8" else qp[ci])
                    nc.sync.dma_start(
                        out=dstp,
                        in_=qf[:, ci, :].rearrange("p (r c) -> p r c", c=32),
                    )

                oim = epi.tile([128, CCH, HW], F32, tag="oim")
                for co in range(CCH):
                    for half in range(2):
                        ps = psum.tile([128, 512], F32, tag="ps")
                        _conv_matmuls(nc, ps, w2s, qp, co, half)
                        sl = slice(half * 512, (half + 1) * 512)
                        nc.vector.scalar_tensor_tensor(
                            out=oim[:, co, sl],
                            in0=ps,
                            scalar=vt[V_A2, co],
                            in1=ht[n, co][:, sl],
                            op0=AOP.mult,
                            op1=AOP.add,
                        )
                nc.sync.dma_start(out=out_d[n].rearrange("c p f -> p c f"), in_=oim)

    nc.finalize()
    _cache["nc"] = nc
    return nc


def _host_prep(x, w1, w2, gamma1, beta1, mean1, var1, gamma2, beta2, mean2, var2):
    f64 = np.float64
    npq = mybir.dt.np(QDT)

    s1 = (gamma1.astype(f64) / np.sqrt(var1.astype(f64) + EPS))
    b1 = beta1.astype(f64) - mean1.astype(f64) * s1
    assert (s1 > 0).all(), "kernel assumes positive bn scale (gamma>0)"
    # delta1 on host (f64 accumulate)
    z1 = x.astype(f64) * s1[None, :, None, None] + b1[None, :, None, None]
    d1 = FRAC * np.abs(z1).mean()
    t1hi = ((d1 - b1) / s1).astype(np.float32)
    t1lo = ((-d1 - b1) / s1).astype(np.float32)

    s2 = (gamma2.astype(f64) / np.sqrt(var2.astype(f64) + EPS))
    b2 = beta2.astype(f64) - mean2.astype(f64) * s2
    assert (s2 > 0).all(), "kernel assumes positive bn scale (gamma>0)"

    a1 = np.abs(w1.astype(f64)).mean(axis=(1, 2, 3)).astype(np.float32)
    a2 = np.abs(w2.astype(f64)).mean(axis=(1, 2, 3)).astype(np.float32)

    def wsign_t(w):
        s = np.sign(w).astype(npq)
        # (O, I, 3, 3) -> (kh, kw, I, O)
        t = s.transpose(2, 3, 1, 0).reshape(9, CCH, 128, C)  # [tap, ci, k, co]
        if QMODE == "fp8":
            t = t.transpose(0, 2, 1, 3)  # [tap, k, ci, co]
        return np.ascontiguousarray(t)

    w1t = wsign_t(w1)
    w2t = wsign_t(w2)

    vecs = np.zeros((NVEC, CCH, 128, 1), np.float32)
    vecs[V_NT1HI] = (-t1hi).reshape(CCH, 128, 1)
    vecs[V_NT1LO] = (-t1lo).reshape(CCH, 128, 1)
    # q values are sign(.)+sign(.) in {-2..2}; fold the /2 into alpha
    vecs[V_A1] = (0.5 * a1).reshape(CCH, 128, 1)
    vecs[V_A2] = (0.5 * a2).reshape(CCH, 128, 1)
    vecs[V_S2] = s2.astype(np.float32).reshape(CCH, 128, 1)
    vecs[V_B2] = b2.astype(np.float32).reshape(CCH, 128, 1)
    # fused threshold coefficients:  -t2hi = b2/s2 - (k*tot)/s2 ;  -t2lo = b2/s2 + (k*tot)/s2
    # with delta2 = k*tot, k = FRAC / (B*C*H*W)
    k = FRAC / float(B * C * HW)
    vecs[V_BRS2] = (b2 / s2).astype(np.float32).reshape(CCH, 128, 1)
    vecs[V_KRS2] = (k / s2).astype(np.float32).reshape(CCH, 128, 1)
    vecs[V_NKRS2] = (-k / s2).astype(np.float32).reshape(CCH, 128, 1)
    return w1t, w2t, vecs


def make_in_maps(**inputs):
    x = np.ascontiguousarray(inputs["x"], np.float32)
    w1t, w2t, vecs = _host_prep(
        x,
        np.asarray(inputs["w1"], np.float32),
        np.asarray(inputs["w2"], np.float32),
        *[np.asarray(inputs[k], np.float32) for k in (
            "gamma1", "beta1", "mean1", "var1",
            "gamma2", "beta2", "mean2", "var2",
        )],
    )
    in_maps = []
    for i in range(NCORES):
        xs = np.ascontiguousarray(
            x[i * BL : (i + 1) * BL].reshape(BL, CCH, 128, HW)
        )
        in_maps.append({"x": xs, "w1t": w1t, "w2t": w2t, "vecs": vecs})
    return in_maps


def kernel(**inputs) -> np.ndarray:
    global LAST_RESULT
    nc = _build()
    in_maps = make_in_maps(**inputs)
    res = run_bass_kernel_spmd(nc, in_maps, list(range(NCORES)), trace=TRACE)
    LAST_RESULT = res
    out = np.concatenate(
        [res.results[i]["out"].reshape(BL, C, H, W) for i in range(NCORES)], axis=0
    )
    return out.astype(np.float32, copy=False)
